# revision 8
# baseline (speedup 1.0000x reference)
"""Trainium2 Bass kernel for nn_Block_86096914416145 (spiking-neuron block).

Reference computation (T=1024, B=32, N=512, fp32):
    m_t = beta * m_{t-1} + c_t           (per-channel leaky integrator, m_{-1}=v_init)
    s_t = (m_t >= v_th)                  (v_th == 1.0 by input contract)
    z   = cumsum(cumsum(s, t), t)        (exact small integers in fp32)
    gz  = (z == 1)                       (forward value of the masked output)
    returns (gz, z, m_last)

Sharding: data-parallel over batch, 4 batches per core across 8 cores.

Per-core dataflow:
    current (T,4,N) --DMA natural [t=128p, n=512f]--> SBUF
      --PE transpose 128x128 blocks--> PSUM [n=128p, t=1024f]
      --DVE tensor_tensor_scan (mult,add)--> m  --gpsimd is_ge--> spikes
      --DVE scan (add)--> c1 --DVE scan (add)--> z   (all [n,t] layout)
    z --PE transpose--> PSUM [t,n] --ACT copy--> SBUF --DMA--> z_out
    zT(PSUM) --ACT Abs(z-1), Relu(1-.)--> gz (natural layout) --DMA--> gz_out
    m[:, -1] --DMA--> m_last
"""

from contextlib import ExitStack

import numpy as np

import concourse.bass as bass
import concourse.tile as tile
from concourse import bacc, mybir
from concourse.bass_utils import run_bass_kernel_spmd

T = 1024
B = 32
N = 512
NCORES = 8
BL = B // NCORES       # batches per core = 4
NGRP = N // 128        # channel groups = 4
NCHUNK = T // 128      # time chunks = 8
FP32 = mybir.dt.float32
Alu = mybir.AluOpType
Act = mybir.ActivationFunctionType

_CACHE = {}


def _build_bass():
    nc = bacc.Bacc("TRN2", target_bir_lowering=False, debug=False)

    cur = nc.dram_tensor("cur", [T, BL, N], FP32, kind="ExternalInput").ap()
    beta2d = nc.dram_tensor("beta2d", [N, 1], FP32, kind="ExternalInput").ap()
    vinitT = nc.dram_tensor("vinitT", [N, BL], FP32, kind="ExternalInput").ap()
    ident = nc.dram_tensor("ident", [128, 128], FP32, kind="ExternalInput").ap()
    z_out = nc.dram_tensor("z_out", [T, BL, N], FP32, kind="ExternalOutput").ap()
    gz_out = nc.dram_tensor("gz_out", [T, BL, N], FP32, kind="ExternalOutput").ap()
    ml_out = nc.dram_tensor("ml_out", [BL, N], FP32, kind="ExternalOutput").ap()

    with ExitStack() as ctx:
        tc = ctx.enter_context(tile.TileContext(nc))
        const = ctx.enter_context(tc.tile_pool(name="const", bufs=1))
        p_cur = ctx.enter_context(tc.tile_pool(name="p_cur", bufs=10))
        p_m = ctx.enter_context(tc.tile_pool(name="p_m", bufs=3))
        p_spk = ctx.enter_context(tc.tile_pool(name="p_spk", bufs=3))
        p_c1 = ctx.enter_context(tc.tile_pool(name="p_c1", bufs=3))
        p_z = ctx.enter_context(tc.tile_pool(name="p_z", bufs=6))
        p_out = ctx.enter_context(tc.tile_pool(name="p_out", bufs=4))
        pp_cur = ctx.enter_context(tc.tile_pool(name="pp_cur", bufs=2, space="PSUM"))
        pp_z = ctx.enter_context(tc.tile_pool(name="pp_z", bufs=2, space="PSUM"))

        ident_sb = const.tile([128, 128], FP32)
        nc.sync.dma_start(ident_sb[:], ident[:])
        neg1 = const.tile([128, 1], FP32)
        nc.gpsimd.memset(neg1[:], -1.0)

        vT = []
        bb = []
        for g in range(NGRP):
            gs = slice(g * 128, (g + 1) * 128)
            vt = const.tile([128, BL], FP32, tag=f"vT{g}")
            nc.sync.dma_start(vt[:], vinitT[gs, :])
            vT.append(vt)
            bcol = const.tile([128, 1], FP32, tag=f"bcol{g}")
            nc.sync.dma_start(bcol[:], beta2d[gs, :])
            # beta replicated along the free (time) dim via a stride-0 AP
            bb.append(bcol[:].to_broadcast((128, T)))

        for b in range(BL):
            cur_nat = []
            for k in range(NCHUNK):
                cn = p_cur.tile([128, N], FP32, tag="cur_nat")
                nc.sync.dma_start(cn[:], cur[k * 128:(k + 1) * 128, b, :])
                cur_nat.append(cn)

            z_sb = []
            for g in range(NGRP):
                gs = slice(g * 128, (g + 1) * 128)
                curT = pp_cur.tile([128, T], FP32, tag="curT")
                for k in range(NCHUNK):
                    nc.tensor.transpose(
                        curT[:, k * 128:(k + 1) * 128], cur_nat[k][:, gs], ident_sb[:]
                    )
                m_t = p_m.tile([128, T], FP32, tag="m")
                nc.vector.tensor_tensor_scan(
                    m_t[:], bb[g], curT[:], vT[g][:, b:b + 1], Alu.mult, Alu.add
                )
                nc.sync.dma_start(ml_out[b, g * 128:(g + 1) * 128], m_t[:, T - 1:T])
                spk = p_spk.tile([128, T], FP32, tag="spk")
                nc.gpsimd.tensor_scalar(spk[:], m_t[:], 1.0, None, Alu.is_ge)
                c1 = p_c1.tile([128, T], FP32, tag="c1")
                nc.vector.tensor_tensor_scan(
                    c1[:], spk[:], spk[:], 0.0, Alu.add, Alu.bypass
                )
                z_t = p_z.tile([128, T], FP32, tag="z")
                nc.vector.tensor_tensor_scan(
                    z_t[:], c1[:], c1[:], 0.0, Alu.add, Alu.bypass
                )
                z_sb.append(z_t)

            for k in range(NCHUNK):
                ks = slice(k * 128, (k + 1) * 128)
                zT = pp_z.tile([128, N], FP32, tag="zT")
                for g in range(NGRP):
                    nc.tensor.transpose(
                        zT[:, g * 128:(g + 1) * 128], z_sb[g][:, ks], ident_sb[:]
                    )
                z_nat = p_out.tile([128, N], FP32, tag="z_nat")
                nc.scalar.activation(z_nat[:], zT[:], Act.Copy)
                nc.sync.dma_start(z_out[ks, b, :], z_nat[:])
                t1 = p_out.tile([128, N], FP32, tag="t1")
                nc.scalar.activation(t1[:], zT[:], Act.Abs, bias=neg1[:])
                gz_nat = p_out.tile([128, N], FP32, tag="gz_nat")
                nc.scalar.activation(gz_nat[:], t1[:], Act.Relu, bias=1.0, scale=-1.0)
                nc.sync.dma_start(gz_out[ks, b, :], gz_nat[:])

    nc.compile()
    return nc


def _get_nc():
    if "nc" not in _CACHE:
        _CACHE["nc"] = _build_bass()
    return _CACHE["nc"]


def _make_in_maps(current, beta, v_init):
    ident = np.eye(128, dtype=np.float32)
    beta2d = np.ascontiguousarray(
        np.asarray(beta, dtype=np.float32).reshape(N, 1)
    )
    in_maps = []
    for c in range(NCORES):
        bsl = slice(c * BL, (c + 1) * BL)
        in_maps.append(
            {
                "cur": np.ascontiguousarray(current[:, bsl, :], dtype=np.float32),
                "beta2d": beta2d,
                "vinitT": np.ascontiguousarray(
                    np.asarray(v_init[bsl, :], dtype=np.float32).T
                ),
                "ident": ident,
            }
        )
    return in_maps


def _numpy_fallback(current, beta, v_init, v_th):
    cur = np.asarray(current, dtype=np.float32)
    beta = np.asarray(beta, dtype=np.float32)
    m = np.asarray(v_init, dtype=np.float32).copy()
    mem = np.empty_like(cur)
    for t in range(cur.shape[0]):
        m = (beta[None, :] * m).astype(np.float32)
        m = (m + cur[t]).astype(np.float32)
        mem[t] = m
    spikes = (mem >= np.asarray(v_th, dtype=np.float32)).astype(np.float32)
    c1 = np.cumsum(spikes, axis=0, dtype=np.float32)
    z = np.cumsum(c1, axis=0, dtype=np.float32)
    gz = (z == 1.0).astype(np.float32)
    return gz, z, mem[-1]


def kernel(current, beta, v_init, v_th):
    current = np.asarray(current)
    beta = np.asarray(beta)
    v_init = np.asarray(v_init)
    v_th = np.asarray(v_th)
    if not (v_th == 1.0).all():
        # The on-device kernel hardcodes the contractual v_th == 1.0.
        return _numpy_fallback(current, beta, v_init, v_th)

    nc = _get_nc()
    in_maps = _make_in_maps(current, beta, v_init)
    res = run_bass_kernel_spmd(nc, in_maps, list(range(NCORES))).results
    z = np.concatenate([res[c]["z_out"] for c in range(NCORES)], axis=1)
    gz = np.concatenate([res[c]["gz_out"] for c in range(NCORES)], axis=1)
    ml = np.concatenate([res[c]["ml_out"] for c in range(NCORES)], axis=0)
    return gz, z, ml
